# revision 5
# baseline (speedup 1.0000x reference)
"""Conv1d (B=32, C_in=C_out=256, W=4096, K=3, pad=1) on 8 Trainium2 cores.

Strategy: data-parallel over batch (4 per core). Per core the conv is a sum
of 6 accumulated matmuls per 512-position output chunk: contraction over
(tap u in 0..2, ci_chunk in 0..1) with lhsT = weight[ci_chunk, :, co_chunk,
u].T ([128 ci x 128 co]) and rhs = a padded-x slice [128 ci x 512]. fp16
inputs (same PE rate as bf16, 8x lower error), fp32 PSUM accumulation, bias
added during the PSUM->SBUF drain (alternating DVE / ACT engines).

Schedule (v2, tuned from the HW trace of v1 which ran 105.3us with the PE
98.8% busy inside its 86.6us matmul span - all the waste was at the edges):
- ~24 warmup matmuls on a memset tile run during the ~6us framework
  prologue so the HAM clock-gate (PE 1.2->2.4 GHz after ~3.4us of activity)
  is warm before the first real matmul.
- first-needed data lands first: weights as two 6-tile DMAs (co-major) plus
  bias on the idle gpsimd queue; batch-0 x as 16 chunk tiles [128,514] with
  the chunk-0 pair leading the sync queue (earliest HWDGE ring start),
  n1..n7 on the scalar queue; batches 1-3 as quarter tiles on vector/gpsimd.
- output staged and stored as fp16 (host upcasts; rel-err ~4e-4 vs 2e-2
  budget): halves store traffic and the tail store.
- last row stores per-chunk and the final chunk's drain is split across
  DVE+ACT so the post-matmul tail is ~1us instead of ~4us.
"""

import numpy as np

F16 = np.float16

B, C, W, K = 32, 256, 4096, 3
NCORES = 8
BPC = B // NCORES          # batches per core
P = 128                    # partitions
CIC = C // P               # ci chunks
COC = C // P               # co chunks
NCH = 512                  # positions per matmul (one PSUM bank of fp32)
NCHUNKS = W // NCH         # position chunks per batch row
NQ = 4                     # x quarter tiles (batches 1-3)
QW = W // NQ               # 1024 positions per quarter
NWARM = 24                 # warmup matmuls (N=128, ~110ns each cold)

_cache = {}


def _build_program():
    import concourse.bass as bass
    import concourse.bacc as bacc
    import concourse.mybir as mybir
    from concourse import tile

    nc = bacc.Bacc(None, target_bir_lowering=False)
    # x, padded by one position on each side: [BPC, CIC, 128, W+2].
    xp_d = nc.dram_tensor("xp", [BPC, CIC, P, W + 2], mybir.dt.float16,
                          kind="ExternalInput")
    # weight, co-major: [ci_in, (co_c, u, ci_c), co_in]
    w_d = nc.dram_tensor("wt", [P, COC * K * CIC, P], mybir.dt.float16,
                         kind="ExternalInput")
    b_d = nc.dram_tensor("bb", [P, COC], mybir.dt.float32,
                         kind="ExternalInput")
    out_d = nc.dram_tensor("out", [BPC, COC, P, W], mybir.dt.float16,
                           kind="ExternalOutput")

    with tile.TileContext(nc) as tc:
        with (
            tc.tile_pool(name="wp", bufs=1) as wp,
            tc.tile_pool(name="xpool", bufs=BPC * CIC * NQ + CIC * NCHUNKS) as xpool,
            tc.tile_pool(name="opool", bufs=3) as opool,
            tc.tile_pool(name="pspool", bufs=8, space=bass.MemorySpace.PSUM) as pspool,
        ):
            # PE warmup: matmuls on a zeroed tile, no DMA dependency, so the
            # HAM activity window fills during the framework prologue.
            warm = wp.tile([P, P], mybir.dt.float16)
            nc.vector.memset(warm[:], 0.0)
            wps = pspool.tile([P, NCH], mybir.dt.float32, name="warm_ps",
                              tag="ps")
            for _ in range(NWARM):
                nc.tensor.matmul(wps[:, 0:P], warm[:], warm[:],
                                 start=True, stop=True)

            # weights: one tile per co chunk (6 lhsT tiles each), bias first.
            b_sb = wp.tile([P, COC], mybir.dt.float32)
            nc.gpsimd.dma_start(b_sb[:], b_d[:])
            w_sb = []
            for co in range(COC):
                wt = wp.tile([P, K * CIC, P], mybir.dt.float16,
                             name=f"w_{co}")
                nc.gpsimd.dma_start(wt[:], w_d[:, co * K * CIC:(co + 1) * K * CIC, :])
                w_sb.append(wt)

            # x tiles. batch 0: per-chunk tiles [128, 514] (chunk n covers
            # padded cols n*512 .. n*512+513). chunk 0 rides the sync queue
            # (earliest ring start), n1-n4 the scalar queue, n5-n7 gpsimd.
            # batches 1-3: per-quarter tiles [128, 1026] on gpsimd (only
            # sync/scalar/gpsimd can issue DMAs).
            x_sb = {}
            for ci in range(CIC):
                for n in range(NCHUNKS):
                    x_sb[(0, ci, n)] = xpool.tile(
                        [P, NCH + 2], mybir.dt.float16, name=f"xc_{ci}_{n}",
                        tag="xc")
            for b in range(1, BPC):
                for ci in range(CIC):
                    for q in range(NQ):
                        x_sb[(b, ci, q)] = xpool.tile(
                            [P, QW + 2], mybir.dt.float16,
                            name=f"xt_{b}_{ci}_{q}", tag="xt")
            for ci in range(CIC):
                nc.sync.dma_start(x_sb[(0, ci, 0)][:],
                                  xp_d[0, ci, :, 0:NCH + 2])
            for n in range(1, NCHUNKS):
                eng = nc.scalar if n <= 4 else nc.gpsimd
                for ci in range(CIC):
                    eng.dma_start(
                        x_sb[(0, ci, n)][:],
                        xp_d[0, ci, :, n * NCH:n * NCH + NCH + 2])
            for b in range(1, BPC):
                for q in range(NQ):
                    for ci in range(CIC):
                        nc.gpsimd.dma_start(
                            x_sb[(b, ci, q)][:],
                            xp_d[b, ci, :, q * QW:q * QW + QW + 2])

            def rhs(b, ci, n, u):
                if b == 0:
                    return x_sb[(0, ci, n)][:, u:u + NCH]
                q = (n * NCH) // QW
                lo = n * NCH + u - q * QW
                return x_sb[(b, ci, q)][:, lo:lo + NCH]

            NACC = K * CIC
            for b in range(BPC):
                for co in range(COC):
                    last_row = (b == BPC - 1 and co == COC - 1)
                    o_sb = opool.tile([P, W], mybir.dt.float16)
                    for n in range(NCHUNKS):
                        ps = pspool.tile([P, NCH], mybir.dt.float32,
                                         name=f"ps_{b}_{co}_{n}", tag="ps")
                        for k, (u, ci) in enumerate(
                                (u, ci) for u in range(K) for ci in range(CIC)):
                            nc.tensor.matmul(
                                ps[:], w_sb[co][:, u * CIC + ci, :],
                                rhs(b, ci, n, u),
                                start=(k == 0), stop=(k == NACC - 1),
                            )
                        dst = o_sb[:, n * NCH:(n + 1) * NCH]
                        if last_row and n == NCHUNKS - 1:
                            # split the final drain across both engines
                            H = NCH // 2
                            nc.scalar.add(dst[:, 0:H], ps[:, 0:H],
                                          b_sb[:, co:co + 1])
                            nc.vector.tensor_scalar_add(
                                dst[:, H:NCH], ps[:, H:NCH],
                                b_sb[:, co:co + 1])
                        elif n % 2 == 0:
                            nc.scalar.add(dst, ps[:], b_sb[:, co:co + 1])
                        else:
                            nc.vector.tensor_scalar_add(
                                dst, ps[:], b_sb[:, co:co + 1])
                        if last_row:
                            nc.sync.dma_start(
                                out_d[b, co, :, n * NCH:(n + 1) * NCH],
                                o_sb[:, n * NCH:(n + 1) * NCH])
                        elif n % 2 == 1:  # flush each finished quarter
                            qq = n // 2
                            nc.sync.dma_start(
                                out_d[b, co, :, qq * QW:(qq + 1) * QW],
                                o_sb[:, qq * QW:(qq + 1) * QW])
    nc.compile()
    return nc


def _prep_inputs(x, weight, bias):
    # x: [32,256,4096] f32 -> padded fp16 [B, CIC, 128, W+2]
    xp = np.zeros((B, CIC, P, W + 2), F16)
    xp[:, :, :, 1:W + 1] = x.reshape(B, CIC, P, W).astype(F16)
    # weight: [co, ci, u] -> [ci_in, (co_c, u, ci_c), co_in]
    wt = weight.reshape(COC, P, CIC, P, K)          # [co_c, co_in, ci_c, ci_in, u]
    w_host = np.ascontiguousarray(
        wt.transpose(3, 0, 4, 2, 1)                 # [ci_in, co_c, u, ci_c, co_in]
    ).reshape(P, COC * K * CIC, P).astype(F16)
    b_host = np.ascontiguousarray(bias.reshape(COC, P).T).astype(np.float32)
    return xp, w_host, b_host


def run(x, weight, bias, trace=False):
    from concourse.bass_utils import run_bass_kernel_spmd

    if "nc" not in _cache:
        _cache["nc"] = _build_program()
    nc = _cache["nc"]

    xp, w_host, b_host = _prep_inputs(
        np.asarray(x, np.float32), np.asarray(weight, np.float32),
        np.asarray(bias, np.float32))
    in_maps = [
        {"xp": xp[c * BPC:(c + 1) * BPC], "wt": w_host, "bb": b_host}
        for c in range(NCORES)
    ]
    res = run_bass_kernel_spmd(nc, in_maps, list(range(NCORES)), trace=trace)
    out = np.concatenate(
        [res.results[c]["out"].reshape(BPC, C, W).astype(np.float32)
         for c in range(NCORES)], axis=0)
    return out, res


def kernel(x, weight, bias):
    out, _ = run(x, weight, bias, trace=False)
    return out
